# revision 1
# baseline (speedup 1.0000x reference)
"""Trainium2 Bass kernel for a Neural CDE (GunnarODE).

Full-input contract: kernel(**inputs) takes the complete (unsharded) inputs
and returns the complete (L, B, Y) output. Internally the batch dim (B=256)
is sharded across 8 NeuronCores (32 per core); the small MLP weights are
replicated. The sequential 127-step x 2-substep Euler scan runs on-device;
spline-derivative terms (z-independent) are folded on the host into
block-diagonal per-substep matrices so the einsum('bhc,bc->bh') contraction
becomes plain PE matmuls.
"""

import sys

for _p in ("/opt/trn_rl_repo", "/root/.axon_site/_ro/trn_rl_repo"):
    if _p not in sys.path:
        sys.path.append(_p)

import numpy as np
import ml_dtypes

import concourse.bass as bass
import concourse.bacc as bacc
import concourse.mybir as mybir
from concourse.tile import TileContext
from concourse.bass_utils import run_bass_kernel_spmd

# Problem dims (hardcoded per contract)
L, B, H, C, Y = 128, 256, 256, 17, 16
N_SUB = 2
NCORES = 8
BC = B // NCORES           # batch per core = 32
NSTEPS = L - 1             # 127 scan steps
NSUBSTEPS = NSTEPS * N_SUB # 254
CP = 20                    # c padded to 5 groups of 4
NG = CP // 4               # 5 contraction groups
F32 = mybir.dt.float32
F32R = mybir.dt.float32r
BF16 = mybir.dt.bfloat16

AluOp = mybir.AluOpType
Act = mybir.ActivationFunctionType


def build_bass(n_substeps=NSUBSTEPS, tanh_split=(512, 1280), dump=False, with_bias2=True):
    """Build the per-core Bass program (same program for all cores)."""
    nc = bacc.Bacc("TRN2", target_bir_lowering=False, debug=False)

    w1s = nc.dram_tensor("w1s", [128, 512], BF16, kind="ExternalInput")
    w2s = nc.dram_tensor("w2s", [128, 2 * CP * 256], BF16, kind="ExternalInput")
    b2s = nc.dram_tensor("b2s", [128, CP * 256], BF16, kind="ExternalInput")
    b1t = nc.dram_tensor("b1t", [128, 2], F32, kind="ExternalInput")
    zt0 = nc.dram_tensor("zt0", [128, 2 * BC], F32, kind="ExternalInput")
    dall = nc.dram_tensor("dall", [n_substeps, 128, NG * BC], F32R, kind="ExternalInput")
    ones = nc.dram_tensor("ones", [128, BC], BF16, kind="ExternalInput")  # bf16 [I;0] selector
    ident = nc.dram_tensor("ident", [BC, BC], F32, kind="ExternalInput")
    decw = nc.dram_tensor("decw", [128, 2 * Y], F32R, kind="ExternalInput")
    decb = nc.dram_tensor("decb", [128, 1], F32, kind="ExternalInput")
    out = nc.dram_tensor("out", [L, BC, Y], F32, kind="ExternalOutput")
    if dump:
        dbg_hdn = nc.dram_tensor("dbg_hdn", [128, 2 * BC], F32, kind="ExternalOutput")
        dbg_g = nc.dram_tensor("dbg_g", [128, NG * 256], F32, kind="ExternalOutput")
        dbg_zd = nc.dram_tensor("dbg_zd", [BC, 256], F32, kind="ExternalOutput")
        dbg_zt = nc.dram_tensor("dbg_zt", [128, 2 * BC], F32, kind="ExternalOutput")

    GF = 256  # free width of one c-group block in G
    with TileContext(nc) as tc:
        with (
            tc.tile_pool(name="const", bufs=1) as constp,
            tc.tile_pool(name="dpool", bufs=4) as dpool,
            tc.tile_pool(name="work", bufs=2) as work,
            tc.tile_pool(name="state", bufs=1) as statep,
            tc.tile_pool(name="psA", bufs=1, space="PSUM") as psA,
            tc.tile_pool(name="psB", bufs=1, space="PSUM") as psB,
        ):
            w1 = constp.tile([128, 512], BF16)
            nc.sync.dma_start(w1[:], w1s[:])
            w2 = constp.tile([128, 2 * CP * 256], BF16)
            nc.sync.dma_start(w2[:], w2s[:])
            b2 = constp.tile([128, CP * 256], BF16)
            nc.sync.dma_start(b2[:], b2s[:])
            b1 = constp.tile([128, 2], F32)
            nc.sync.dma_start(b1[:], b1t[:])
            onest = constp.tile([128, BC], BF16)
            nc.sync.dma_start(onest[:], ones[:])
            identt = constp.tile([BC, BC], F32)
            nc.sync.dma_start(identt[:], ident[:])
            decwt = constp.tile([128, 2 * Y], F32R)
            nc.sync.dma_start(decwt[:], decw[:])
            decbt = constp.tile([128, 1], F32)
            nc.sync.dma_start(decbt[:], decb[:])

            zT = statep.tile([128, 2 * BC], F32)   # fp32 master state, transposed
            nc.sync.dma_start(zT[:], zt0[:])
            zTr = statep.tile([128, 2 * BC], BF16)  # rounded copy fed to matmuls
            nc.vector.tensor_copy(zTr[:], zT[:])
            zsT = statep.tile([128, L * 2 * BC], F32R)  # all L z-snapshots, transposed
            nc.vector.tensor_copy(zsT[:, 0 : 2 * BC], zT[:])

            for k in range(n_substeps):
                dk = dpool.tile([128, NG * BC], F32R, tag="dk")
                nc.sync.dma_start(dk[:], dall[k])

                # ---- GEMM1: hdnT = (z @ W1)^T via col-tiled strips ----
                hdnP = psB.tile([128, 2 * BC], F32, tag="hdnP")
                for mh in range(2):
                    for kh in range(2):
                        for jj in range(4):
                            nc.tensor.matmul(
                                hdnP[32 * jj : 32 * jj + 32, mh * BC : (mh + 1) * BC],
                                w1[:, (kh * 2 + mh) * 128 + 32 * jj : (kh * 2 + mh) * 128 + 32 * jj + 32],
                                zTr[:, kh * BC : (kh + 1) * BC],
                                start=(kh == 0),
                                stop=(kh == 1),
                                tile_position=(0, 32 * jj),
                            )
                # relu(x + b1), per h-half (bias is per-partition within a half)
                hdn = work.tile([128, 2 * BC], BF16, tag="hdn")
                for mh in range(2):
                    nc.vector.tensor_scalar(
                        hdn[:, mh * BC : (mh + 1) * BC],
                        hdnP[:, mh * BC : (mh + 1) * BC],
                        b1[:, mh : mh + 1],
                        0.0,
                        AluOp.add,
                        AluOp.max,
                    )

                # ---- GEMM2: G = hdn @ W2c + b2 (c-major, 4 c's per psum tile) ----
                gP = psA.tile([128, NG * GF], F32, tag="gP")
                gS = work.tile([128, NG * GF], F32R, tag="gS")
                zdP = psB.tile([BC, 256], F32, tag="zdP")

                tanh_done = 0
                tanh_cuts = list(tanh_split)

                def emit_bias(g):
                    for jj in range(4):
                        c = 4 * g + jj
                        nc.tensor.matmul(
                            gP[32 * jj : 32 * jj + 32, g * GF : (g + 1) * GF],
                            onest[:],
                            b2[:, c * 256 : (c + 1) * 256],
                            start=True,
                            stop=False,
                            tile_position=(0, 32 * jj),
                            skip_group_check=True,
                        )

                def emit_g(g):
                    for kh in range(2):
                        for jj in range(4):
                            c = 4 * g + jj
                            nc.tensor.matmul(
                                gP[32 * jj : 32 * jj + 32, g * GF : (g + 1) * GF],
                                hdn[:, kh * BC : (kh + 1) * BC],
                                w2[:, kh * CP * 256 + c * 256 : kh * CP * 256 + c * 256 + 256],
                                start=(kh == 0 and not with_bias2),
                                stop=(kh == 1),
                                tile_position=(0, 32 * jj),
                                skip_group_check=True,
                            )

                def flush_tanh_and_contract(upto):
                    nonlocal tanh_done
                    nc.scalar.activation(gS[:, tanh_done:upto], gP[:, tanh_done:upto], Act.Tanh)
                    g_lo, g_hi = tanh_done // GF, upto // GF
                    for g in range(g_lo, g_hi):
                        nc.tensor.matmul(
                            zdP[:],
                            dk[:, g * BC : (g + 1) * BC],
                            gS[:, g * GF : (g + 1) * GF],
                            start=(g == 0),
                            stop=(g == NG - 1),
                            skip_group_check=True,
                        )
                    tanh_done = upto

                for g in range(NG):
                    if with_bias2:
                        emit_bias(g)
                    emit_g(g)
                    while tanh_cuts and (g + 1) * GF >= tanh_cuts[0]:
                        flush_tanh_and_contract(tanh_cuts.pop(0))

                # ---- z update: zT += transpose(zdelta) ----
                zd = work.tile([BC, 256], F32, tag="zd")
                nc.vector.tensor_copy(zd[:], zdP[:])
                zdT = psB.tile([128, 2 * BC], F32, tag="zdT")
                for hh in range(2):
                    nc.tensor.transpose(
                        zdT[:, hh * BC : (hh + 1) * BC],
                        zd[:, hh * 128 : (hh + 1) * 128],
                        identt[:],
                    )
                nc.vector.tensor_add(zT[:], zT[:], zdT[:])
                nc.vector.tensor_copy(zTr[:], zT[:])

                if dump and k == 0:
                    dbg_hdn_s = work.tile([128, 2 * BC], F32, tag="dbg1")
                    nc.vector.tensor_copy(dbg_hdn_s[:], hdn[:])
                    nc.sync.dma_start(dbg_hdn[:], dbg_hdn_s[:])
                    dbg_g_s = work.tile([128, NG * 256], F32, tag="dbg2")
                    nc.vector.tensor_copy(dbg_g_s[:], gS[:])
                    nc.sync.dma_start(dbg_g[:], dbg_g_s[:])
                    nc.sync.dma_start(dbg_zd[:], zd[:])
                    nc.sync.dma_start(dbg_zt[:], zT[:])

                if k % 2 == 1:
                    step = k // 2
                    nc.vector.tensor_copy(
                        zsT[:, (step + 1) * 2 * BC : (step + 2) * 2 * BC], zT[:]
                    )

        # ---- decode: out[l, b, y] = zs[l, b, :] @ dec_W + dec_b ----
        with (
            tc.tile_pool(name="psD", bufs=1, space="PSUM") as psD,
            tc.tile_pool(name="od", bufs=1) as odp,
        ):
            zs3 = zsT[:].rearrange("p (e x) -> p e x", x=2 * BC)
            outP = psD.tile([Y, 4096], F32)
            n_sc = L // 8  # 16 step-chunks of 8 entries
            for sc in range(n_sc):
                for hh in range(2):
                    nc.tensor.matmul(
                        outP[:, sc * 256 : (sc + 1) * 256],
                        decwt[:, hh * Y : (hh + 1) * Y],
                        zs3[:, sc * 8 : (sc + 1) * 8, hh * BC : (hh + 1) * BC],
                        start=(hh == 0),
                        stop=(hh == 1),
                        skip_group_check=True,
                    )
            outS = odp.tile([Y, 4096], F32)
            nc.vector.tensor_scalar(
                outS[:], outP[:], decbt[0:Y, 0:1], None, AluOp.add
            )
            outv = out[:].rearrange("(sc s) b y -> sc y s b", s=8)
            for sc in range(n_sc):
                src_ap = outS[:, sc * 256 : (sc + 1) * 256]
                nc.sync.dma_start(outv[sc], src_ap)

    nc.compile()
    return nc


def host_prep(ts, us, enc_b, f_W1, f_b1, f_W2, f_b2, dec_W, dec_b, n_substeps=NSUBSTEPS):
    """Host-side packing of weights + spline-derivative block-diag matrices."""
    ts = np.asarray(ts, np.float64)
    us = np.asarray(us, np.float64)
    t = ts[:, 0, 0]
    dt = t[1:] - t[:-1]                                  # (L-1,)
    x = np.concatenate([ts, us], axis=-1).transpose(1, 0, 2)  # (B, L, C)
    h = dt[None, :, None]
    slope = (x[:, 1:] - x[:, :-1]) / h
    m = np.concatenate([slope[:, :1], slope], axis=1)
    mi, mn = m[:, :-1], m[:, 1:]
    xi, xn = x[:, :-1], x[:, 1:]
    c2 = 3.0 * (xn - xi) / h**2 - (2.0 * mi + mn) / h
    c3 = 2.0 * (xi - xn) / h**3 + (mi + mn) / h**2
    dX0 = mi                                             # u = 0
    dX1 = mi + c2 * h + 0.75 * c3 * h * h                # u = h/2
    scale = h / N_SUB                                    # (1, L-1, 1)
    dxs = np.stack([dX0 * scale, dX1 * scale], axis=2)   # (B, L-1, 2, C)
    dxs = dxs.transpose(1, 2, 0, 3).reshape(NSUBSTEPS, B, C).astype(np.float32)

    f_W1 = np.asarray(f_W1, np.float32)
    f_W2 = np.asarray(f_W2, np.float32)
    f_b1 = np.asarray(f_b1, np.float32)
    f_b2 = np.asarray(f_b2, np.float32)
    enc_b = np.asarray(enc_b, np.float32)
    dec_W = np.asarray(dec_W, np.float32)
    dec_b = np.asarray(dec_b, np.float32)

    # W1 packed: w1s[p, (kh*2+mh)*128 + m] = W1[kh*128+p, mh*128+m]
    w1s = np.zeros((128, 512), np.float32)
    for kh in range(2):
        for mh in range(2):
            w1s[:, (kh * 2 + mh) * 128 : (kh * 2 + mh + 1) * 128] = f_W1[
                kh * 128 : (kh + 1) * 128, mh * 128 : (mh + 1) * 128
            ]

    # W2 c-major padded: w2s[p, kh*5120 + c*256 + h2] = W2[kh*128+p, h2*C + c]
    w2r = f_W2.reshape(H, H, C)                          # [h_in, h_out, c]
    w2cm = np.zeros((H, CP, H), np.float32)
    w2cm[:, :C, :] = w2r.transpose(0, 2, 1)              # [h_in, c, h_out]
    w2cm = w2cm.reshape(H, CP * H)
    w2s = np.concatenate([w2cm[:128], w2cm[128:]], axis=1)  # (128, 2*5120)

    b2r = f_b2.reshape(H, C)
    b2cm = np.zeros((CP, H), np.float32)
    b2cm[:C] = b2r.T
    b2s = np.broadcast_to(b2cm.reshape(1, CP * H), (128, CP * H)).copy()

    b1t = np.stack([f_b1[:128], f_b1[128:]], axis=1).astype(np.float32)  # (128, 2)

    z0 = enc_b                                            # zeros @ enc_W + enc_b
    zt0 = np.zeros((128, 2 * BC), np.float32)
    for hh in range(2):
        zt0[:, hh * BC : (hh + 1) * BC] = z0[hh * 128 : (hh + 1) * 128][:, None]

    # Block-diag dX matrices: dall[k, 32*jj + bb, g*BC + bb] = dxs[k, b0+bb, 4g+jj]
    dall_cores = []
    bb = np.arange(BC)
    for core in range(NCORES):
        d = np.zeros((n_substeps, 4, BC, NG, BC), np.float32)
        for g in range(NG):
            for jj in range(4):
                c = 4 * g + jj
                if c < C:
                    d[:, jj, bb, g, bb] = dxs[:n_substeps, core * BC + bb, c]
        dall_cores.append(d.reshape(n_substeps, 128, NG * BC))

    decw = np.concatenate([dec_W[:128], dec_W[128:]], axis=1).astype(np.float32)  # (128, 2Y)
    decb = np.zeros((128, 1), np.float32)
    for jj in range(4):
        decb[32 * jj : 32 * jj + Y, 0] = dec_b

    common = {
        "w1s": w1s.astype(ml_dtypes.bfloat16),
        "w2s": w2s.astype(ml_dtypes.bfloat16),
        "b2s": b2s.astype(ml_dtypes.bfloat16),
        "b1t": b1t,
        "zt0": zt0,
        "ones": np.eye(128, BC, dtype=ml_dtypes.bfloat16),
        "ident": np.eye(BC, dtype=np.float32),
        "decw": decw,
        "decb": decb,
    }
    in_maps = []
    for core in range(NCORES):
        m_ = dict(common)
        m_["dall"] = dall_cores[core]
        in_maps.append(m_)
    return in_maps


_CACHE = {}


def _get_nc(n_substeps=NSUBSTEPS, dump=False, with_bias2=True):
    key = (n_substeps, dump, with_bias2)
    if key not in _CACHE:
        _CACHE[key] = build_bass(n_substeps, dump=dump, with_bias2=with_bias2)
    return _CACHE[key]


def run(inputs, n_substeps=NSUBSTEPS, trace=False, dump=False, **kw):
    in_maps = host_prep(
        inputs["ts"], inputs["us"], inputs["enc_b"], inputs["f_W1"],
        inputs["f_b1"], inputs["f_W2"], inputs["f_b2"], inputs["dec_W"],
        inputs["dec_b"], n_substeps=n_substeps,
    )
    nc = _get_nc(n_substeps, dump)
    res = run_bass_kernel_spmd(nc, in_maps, core_ids=list(range(NCORES)), trace=trace, **kw)
    outs = [np.asarray(res.results[i]["out"]) for i in range(NCORES)]
    full = np.concatenate(outs, axis=1)  # (L, B, Y)
    return full, res


def kernel(**inputs) -> np.ndarray:
    full, _ = run(inputs)
    return full.astype(np.float32)



# revision 2
# speedup vs baseline: 1.0765x; 1.0765x over previous
"""Trainium2 Bass kernel for a Neural CDE (GunnarODE).

Full-input contract: kernel(**inputs) takes the complete (unsharded) inputs
and returns the complete (L, B, Y) output. Internally the batch dim (B=256)
is sharded across 8 NeuronCores (32 per core); the small MLP weights are
replicated. The sequential 127-step x 2-substep Euler scan runs on-device.

v2 design notes (vs the earlier baseline):
- No per-substep DMA: the spline-derivative scalars for all 254 substeps are
  resident in SBUF as a dense (128, 254*5) tile; the block-diagonal dX
  operand for the contraction matmuls is rebuilt on-device each substep with
  five small DVE tensor_scalar ops (mask * per-partition scalar).
- Channels are not padded (17 groups-of-4 -> [4,4,4,4,1]), cutting the big
  GEMM and bias streams by 15%.
- tanh output gS, dX, the z snapshots and the decoder run in bf16; the
  master state z stays fp32. The rounded bf16 state fed to matmuls is
  produced in a single DVE op (fp32 z + PSUM delta -> bf16), with the fp32
  master update moved off the critical path.
- A ~4.5us dummy-matmul warmup burst precedes the scan so the PE HAM clock
  gate reaches 2.4 GHz before the steady state.
"""

import sys

for _p in ("/opt/trn_rl_repo", "/root/.axon_site/_ro/trn_rl_repo"):
    if _p not in sys.path:
        sys.path.append(_p)

import numpy as np
import ml_dtypes

import concourse.bass as bass
import concourse.bacc as bacc
import concourse.mybir as mybir
from concourse.tile import TileContext
from concourse.bass_utils import run_bass_kernel_spmd

# Problem dims (hardcoded per contract)
L, B, H, C, Y = 128, 256, 256, 17, 16
N_SUB = 2
NCORES = 8
BC = B // NCORES           # batch per core = 32
NSTEPS = L - 1             # 127 scan steps
NSUBSTEPS = NSTEPS * N_SUB # 254
NG = 5                     # c-groups: [4,4,4,4,1]
GF = 256                   # free width of one c-group block in G
F32 = mybir.dt.float32
F32R = mybir.dt.float32r
BF16 = mybir.dt.bfloat16

AluOp = mybir.AluOpType
Act = mybir.ActivationFunctionType


def build_bass(n_substeps=NSUBSTEPS, tanh_cuts=(512, 1024, 1280), warmup=40,
               dump=False):
    """Build the per-core Bass program (same program for all cores)."""
    nc = bacc.Bacc("TRN2", target_bir_lowering=False, debug=False)

    w1s = nc.dram_tensor("w1s", [128, 512], BF16, kind="ExternalInput")
    w2s = nc.dram_tensor("w2s", [128, 2 * C * 256], BF16, kind="ExternalInput")
    b2s = nc.dram_tensor("b2s", [128, C * 256], BF16, kind="ExternalInput")
    b1t = nc.dram_tensor("b1t", [128, 2], F32, kind="ExternalInput")
    zt0 = nc.dram_tensor("zt0", [128, 2 * BC], F32, kind="ExternalInput")
    dcol = nc.dram_tensor("dcol", [128, NSUBSTEPS * NG], F32, kind="ExternalInput")
    maskd = nc.dram_tensor("maskd", [128, NG * BC], BF16, kind="ExternalInput")
    ones = nc.dram_tensor("ones", [128, BC], BF16, kind="ExternalInput")
    ident = nc.dram_tensor("ident", [BC, 128], F32, kind="ExternalInput")
    decw = nc.dram_tensor("decw", [128, 2 * Y], BF16, kind="ExternalInput")
    decb = nc.dram_tensor("decb", [128, 1], F32, kind="ExternalInput")
    out = nc.dram_tensor("out", [L, BC, Y], F32, kind="ExternalOutput")
    if dump:
        dbg_hdn = nc.dram_tensor("dbg_hdn", [128, 2 * BC], F32, kind="ExternalOutput")
        dbg_g = nc.dram_tensor("dbg_g", [128, NG * GF], F32, kind="ExternalOutput")
        dbg_zd = nc.dram_tensor("dbg_zd", [BC, 256], F32, kind="ExternalOutput")
        dbg_zt = nc.dram_tensor("dbg_zt", [128, 2 * BC], F32, kind="ExternalOutput")

    with TileContext(nc) as tc:
        with (
            tc.tile_pool(name="const", bufs=1) as constp,
            tc.tile_pool(name="work", bufs=2) as work,
            tc.tile_pool(name="state", bufs=1) as statep,
            tc.tile_pool(name="psG", bufs=1, space="PSUM") as psG,
            tc.tile_pool(name="psH", bufs=1, space="PSUM") as psH,
            tc.tile_pool(name="psZ", bufs=1, space="PSUM") as psZ,
        ):
            w1 = constp.tile([128, 512], BF16)
            nc.sync.dma_start(w1[:], w1s[:])
            w2 = constp.tile([128, 2 * C * 256], BF16)
            nc.sync.dma_start(w2[:], w2s[:])
            b2 = constp.tile([128, C * 256], BF16)
            nc.sync.dma_start(b2[:], b2s[:])
            b1 = constp.tile([128, 2], F32)
            nc.sync.dma_start(b1[:], b1t[:])
            onest = constp.tile([128, BC], BF16)
            nc.sync.dma_start(onest[:], ones[:])
            identt = constp.tile([BC, 128], F32)
            nc.sync.dma_start(identt[:], ident[:])
            maskt = constp.tile([128, NG * BC], BF16)
            nc.sync.dma_start(maskt[:], maskd[:])
            dcolt = constp.tile([128, NSUBSTEPS * NG], F32)
            nc.sync.dma_start(dcolt[:], dcol[:])
            decwt = constp.tile([128, 2 * Y], BF16)
            nc.sync.dma_start(decwt[:], decw[:])
            decbt = constp.tile([128, 1], F32)
            nc.sync.dma_start(decbt[:], decb[:])

            zT = statep.tile([128, 2 * BC], F32)   # fp32 master state, transposed
            nc.sync.dma_start(zT[:], zt0[:])
            zTr = statep.tile([128, 2 * BC], BF16)  # rounded copy fed to matmuls
            nc.vector.tensor_copy(zTr[:], zT[:])
            zsT = statep.tile([128, L * 2 * BC], BF16)  # all L z-snapshots
            nc.vector.tensor_copy(zsT[:, 0 : 2 * BC], zT[:])

            # ---- HAM warmup: ~40 N=256 dummy matmuls keeps PE busy ~4us so
            # the clock gate opens to 2.4 GHz before the scan starts.
            if warmup:
                wuP = psG.tile([128, NG * GF], F32, tag="gP")
                for i in range(warmup):
                    nc.tensor.matmul(
                        wuP[0:BC, 0:GF],
                        onest[:],
                        w2[:, 0:GF],
                        start=True,
                        stop=True,
                        skip_group_check=True,
                    )

            for k in range(n_substeps):
                # ---- dk: block-diag dX built from resident scalars ----
                dk = work.tile([128, NG * BC], BF16, tag="dk")
                for g in range(NG):
                    nc.vector.tensor_scalar(
                        dk[:, g * BC : (g + 1) * BC],
                        maskt[:, g * BC : (g + 1) * BC],
                        dcolt[:, k * NG + g : k * NG + g + 1],
                        None,
                        AluOp.mult,
                    )

                # ---- GEMM1: hdnT = (z @ W1)^T via col-tiled strips ----
                hdnP = psH.tile([128, 2 * BC], F32, tag="hdnP")
                for mh in range(2):
                    for kh in range(2):
                        for jj in range(4):
                            nc.tensor.matmul(
                                hdnP[32 * jj : 32 * jj + 32, mh * BC : (mh + 1) * BC],
                                w1[:, (kh * 2 + mh) * 128 + 32 * jj : (kh * 2 + mh) * 128 + 32 * jj + 32],
                                zTr[:, kh * BC : (kh + 1) * BC],
                                start=(kh == 0),
                                stop=(kh == 1),
                                tile_position=(0, 32 * jj),
                            )
                # relu(x + b1), per h-half (bias is per-partition within a half)
                hdn = work.tile([128, 2 * BC], BF16, tag="hdn")
                for mh in range(2):
                    nc.vector.tensor_scalar(
                        hdn[:, mh * BC : (mh + 1) * BC],
                        hdnP[:, mh * BC : (mh + 1) * BC],
                        b1[:, mh : mh + 1],
                        0.0,
                        AluOp.add,
                        AluOp.max,
                    )

                # ---- GEMM2: G = b2 + hdn @ W2c (c-major groups [4,4,4,4,1]) ----
                gP = psG.tile([128, NG * GF], F32, tag="gP")
                gS = work.tile([128, NG * GF], BF16, tag="gS")
                zdP = psZ.tile([BC, 256], F32, tag="zdP")

                def n_jj(g):
                    return 4 if g < 4 else 1

                def emit_bias(g):
                    for jj in range(n_jj(g)):
                        c = 4 * g + jj
                        nc.tensor.matmul(
                            gP[32 * jj : 32 * jj + 32, g * GF : (g + 1) * GF],
                            onest[:],
                            b2[:, c * 256 : (c + 1) * 256],
                            start=True,
                            stop=False,
                            tile_position=(0, 32 * jj),
                            skip_group_check=True,
                        )

                def emit_g(g):
                    for kh in range(2):
                        for jj in range(n_jj(g)):
                            c = 4 * g + jj
                            nc.tensor.matmul(
                                gP[32 * jj : 32 * jj + 32, g * GF : (g + 1) * GF],
                                hdn[:, kh * BC : (kh + 1) * BC],
                                w2[:, kh * C * 256 + c * 256 : kh * C * 256 + c * 256 + 256],
                                start=False,
                                stop=(kh == 1),
                                tile_position=(0, 32 * jj),
                                skip_group_check=True,
                            )

                tanh_done = 0
                cuts = list(tanh_cuts)

                def flush_tanh_and_contract(upto):
                    nonlocal tanh_done
                    nc.scalar.activation(gS[:, tanh_done:upto], gP[:, tanh_done:upto], Act.Tanh)
                    g_lo, g_hi = tanh_done // GF, upto // GF
                    for g in range(g_lo, g_hi):
                        if g < 4:
                            nc.tensor.matmul(
                                zdP[:],
                                dk[:, g * BC : (g + 1) * BC],
                                gS[:, g * GF : (g + 1) * GF],
                                start=(g == 0),
                                stop=False,
                                skip_group_check=True,
                            )
                        else:  # singleton c16 group lives on partitions 0:32
                            nc.tensor.matmul(
                                zdP[:],
                                dk[0:BC, g * BC : (g + 1) * BC],
                                gS[0:BC, g * GF : (g + 1) * GF],
                                start=False,
                                stop=True,
                                skip_group_check=True,
                            )
                    tanh_done = upto

                for g in range(NG):
                    emit_bias(g)
                for g in range(NG):
                    emit_g(g)
                    while cuts and (g + 1) * GF >= cuts[0]:
                        flush_tanh_and_contract(cuts.pop(0))

                # ---- z update: zT += transpose(zdelta) ----
                zd = work.tile([BC, 256], F32, tag="zd")
                nc.vector.tensor_copy(zd[:], zdP[:])
                zdT = psZ.tile([128, 2 * BC], F32, tag="zdT")
                for hh in range(2):
                    nc.tensor.transpose(
                        zdT[:, hh * BC : (hh + 1) * BC],
                        zd[:, hh * 128 : (hh + 1) * 128],
                        identt[:, 0:BC],
                    )
                # rounded state first (critical path), fp32 master off-path
                nc.vector.tensor_add(zTr[:], zT[:], zdT[:])
                nc.vector.tensor_add(zT[:], zT[:], zdT[:])

                if dump and k == 0:
                    dbg_hdn_s = work.tile([128, 2 * BC], F32, tag="dbg1")
                    nc.vector.tensor_copy(dbg_hdn_s[:], hdn[:])
                    nc.sync.dma_start(dbg_hdn[:], dbg_hdn_s[:])
                    dbg_g_s = work.tile([128, NG * GF], F32, tag="dbg2")
                    nc.vector.tensor_copy(dbg_g_s[:], gS[:])
                    nc.sync.dma_start(dbg_g[:], dbg_g_s[:])
                    nc.sync.dma_start(dbg_zd[:], zd[:])
                    nc.sync.dma_start(dbg_zt[:], zT[:])

                if k % 2 == 1:
                    step = k // 2
                    nc.vector.tensor_copy(
                        zsT[:, (step + 1) * 2 * BC : (step + 2) * 2 * BC], zTr[:]
                    )

        # ---- decode: out[l, b, y] = zs[l, b, :] @ dec_W + dec_b ----
        with (
            tc.tile_pool(name="psD", bufs=1, space="PSUM") as psD,
            tc.tile_pool(name="od", bufs=1) as odp,
        ):
            zs3 = zsT[:].rearrange("p (e x) -> p e x", x=2 * BC)
            outP = psD.tile([Y, 4096], F32)
            n_sc = L // 8  # 16 step-chunks of 8 entries
            for sc in range(n_sc):
                for hh in range(2):
                    nc.tensor.matmul(
                        outP[:, sc * 256 : (sc + 1) * 256],
                        decwt[:, hh * Y : (hh + 1) * Y],
                        zs3[:, sc * 8 : (sc + 1) * 8, hh * BC : (hh + 1) * BC],
                        start=(hh == 0),
                        stop=(hh == 1),
                        skip_group_check=True,
                    )
            outS = odp.tile([Y, 4096], F32)
            nc.vector.tensor_scalar(
                outS[:], outP[:], decbt[0:Y, 0:1], None, AluOp.add
            )
            outv = out[:].rearrange("(sc s) b y -> sc y s b", s=8)
            for sc in range(n_sc):
                src_ap = outS[:, sc * 256 : (sc + 1) * 256]
                nc.sync.dma_start(outv[sc], src_ap)

    nc.compile()
    return nc


def host_prep(ts, us, enc_b, f_W1, f_b1, f_W2, f_b2, dec_W, dec_b, n_substeps=NSUBSTEPS):
    """Host-side packing of weights + spline-derivative scalars."""
    ts = np.asarray(ts, np.float64)
    us = np.asarray(us, np.float64)
    t = ts[:, 0, 0]
    dt = t[1:] - t[:-1]                                  # (L-1,)
    x = np.concatenate([ts, us], axis=-1).transpose(1, 0, 2)  # (B, L, C)
    h = dt[None, :, None]
    slope = (x[:, 1:] - x[:, :-1]) / h
    m = np.concatenate([slope[:, :1], slope], axis=1)
    mi, mn = m[:, :-1], m[:, 1:]
    xi, xn = x[:, :-1], x[:, 1:]
    c2 = 3.0 * (xn - xi) / h**2 - (2.0 * mi + mn) / h
    c3 = 2.0 * (xi - xn) / h**3 + (mi + mn) / h**2
    dX0 = mi                                             # u = 0
    dX1 = mi + c2 * h + 0.75 * c3 * h * h                # u = h/2
    scale = h / N_SUB                                    # (1, L-1, 1)
    dxs = np.stack([dX0 * scale, dX1 * scale], axis=2)   # (B, L-1, 2, C)
    dxs = dxs.transpose(1, 2, 0, 3).reshape(NSUBSTEPS, B, C).astype(np.float32)

    f_W1 = np.asarray(f_W1, np.float32)
    f_W2 = np.asarray(f_W2, np.float32)
    f_b1 = np.asarray(f_b1, np.float32)
    f_b2 = np.asarray(f_b2, np.float32)
    enc_b = np.asarray(enc_b, np.float32)
    dec_W = np.asarray(dec_W, np.float32)
    dec_b = np.asarray(dec_b, np.float32)

    # W1 packed: w1s[p, (kh*2+mh)*128 + m] = W1[kh*128+p, mh*128+m]
    w1s = np.zeros((128, 512), np.float32)
    for kh in range(2):
        for mh in range(2):
            w1s[:, (kh * 2 + mh) * 128 : (kh * 2 + mh + 1) * 128] = f_W1[
                kh * 128 : (kh + 1) * 128, mh * 128 : (mh + 1) * 128
            ]

    # W2 c-major (no padding): w2s[p, kh*C*256 + c*256 + h2] = W2[kh*128+p, h2*C + c]
    w2r = f_W2.reshape(H, H, C)                          # [h_in, h_out, c]
    w2cm = w2r.transpose(0, 2, 1).reshape(H, C * H)      # [h_in, c, h_out]
    w2s = np.concatenate([w2cm[:128], w2cm[128:]], axis=1)  # (128, 2*C*256)

    b2r = f_b2.reshape(H, C)
    b2cm = b2r.T.reshape(1, C * H)                       # [c, h_out]
    b2s = np.broadcast_to(b2cm, (128, C * H)).copy()

    b1t = np.stack([f_b1[:128], f_b1[128:]], axis=1).astype(np.float32)  # (128, 2)

    z0 = enc_b                                            # zeros @ enc_W + enc_b
    zt0 = np.zeros((128, 2 * BC), np.float32)
    for hh in range(2):
        zt0[:, hh * BC : (hh + 1) * BC] = z0[hh * 128 : (hh + 1) * 128][:, None]

    # mask[32*jj + bb, g*BC + bb'] = (bb == bb') for groups with c = 4g+jj < C
    maskd = np.zeros((128, NG * BC), np.float32)
    bb = np.arange(BC)
    for g in range(NG):
        for jj in range(4 if g < 4 else 1):
            maskd[32 * jj + bb, g * BC + bb] = 1.0

    # dcol[32*jj + bb, k*NG + g] = dxs[k, core*BC + bb, 4g+jj]
    dcol_cores = []
    for core in range(NCORES):
        d = np.zeros((128, NSUBSTEPS * NG), np.float32)
        for g in range(NG):
            for jj in range(4 if g < 4 else 1):
                c = 4 * g + jj
                for ks in range(n_substeps):
                    d[32 * jj + bb, ks * NG + g] = dxs[ks, core * BC + bb, c]
        dcol_cores.append(d)

    decw = np.concatenate([dec_W[:128], dec_W[128:]], axis=1).astype(np.float32)  # (128, 2Y)
    decb = np.zeros((128, 1), np.float32)
    for jj in range(4):
        decb[32 * jj : 32 * jj + Y, 0] = dec_b

    common = {
        "w1s": w1s.astype(ml_dtypes.bfloat16),
        "w2s": w2s.astype(ml_dtypes.bfloat16),
        "b2s": b2s.astype(ml_dtypes.bfloat16),
        "b1t": b1t,
        "zt0": zt0,
        "maskd": maskd.astype(ml_dtypes.bfloat16),
        "ones": np.eye(128, BC, dtype=ml_dtypes.bfloat16),
        "ident": np.eye(BC, 128, dtype=np.float32),
        "decw": decw.astype(ml_dtypes.bfloat16),
        "decb": decb,
    }
    in_maps = []
    for core in range(NCORES):
        m_ = dict(common)
        m_["dcol"] = dcol_cores[core]
        in_maps.append(m_)
    return in_maps


_CACHE = {}


def _get_nc(n_substeps=NSUBSTEPS, dump=False, tanh_cuts=(512, 1024, 1280)):
    key = (n_substeps, dump, tanh_cuts)
    if key not in _CACHE:
        _CACHE[key] = build_bass(n_substeps, tanh_cuts=tanh_cuts, dump=dump)
    return _CACHE[key]


def run(inputs, n_substeps=NSUBSTEPS, trace=False, dump=False,
        tanh_cuts=(512, 1024, 1280), **kw):
    in_maps = host_prep(
        inputs["ts"], inputs["us"], inputs["enc_b"], inputs["f_W1"],
        inputs["f_b1"], inputs["f_W2"], inputs["f_b2"], inputs["dec_W"],
        inputs["dec_b"], n_substeps=n_substeps,
    )
    nc = _get_nc(n_substeps, dump, tanh_cuts)
    res = run_bass_kernel_spmd(nc, in_maps, core_ids=list(range(NCORES)), trace=trace, **kw)
    outs = [np.asarray(res.results[i]["out"]) for i in range(NCORES)]
    full = np.concatenate(outs, axis=1)  # (L, B, Y)
    return full, res


def kernel(**inputs) -> np.ndarray:
    full, _ = run(inputs)
    return full.astype(np.float32)
